# revision 13
# baseline (speedup 1.0000x reference)
"""GRU cell (B=4096, H=2048) on 8 TRN2 NeuronCores.

Sharding: data-parallel over the batch dim — each core computes 512 rows.
Weights are replicated; no collectives.

Per-core compute runs in "transposed" space (hidden on partitions, batch on
the free dim). Precision strategy (gate rel-err < 2e-2; 1.57e-2 in numpy
simulation of this exact scheme):
  - r/z gates and the n-gate's hh half (gh2): fp8-e4m3 DoubleRow matmuls
    (2 contraction rows per PE cell, 2x MAC rate). Acts scaled by SX=32,
    weights by SW=8192 to sit in e4m3's normal range. Sigmoid squashes the
    r/z error; gh2's error is damped by the multiply with r in (0,1).
  - n-gate ih half (gi2): bf16 (its error hits tanh 1:1, fp8 would blow
    the budget).
This cuts weight DMA ~3x and takes the matmul stream to the fp8 roofline
for 5 of 6 K-sweeps.

Schedule: phase S runs all 512 r/z DoubleRow matmuls back-to-back (the
bf16<->fp8 mode switch costs ~0.8us per junction, so junctions are
minimized); sigmoids land in SBUF as bf16. Phase M processes quads of 4
hidden blocks: 4x16 bf16 gi matmuls, then 4x8 DoubleRow gh2 matmuls (one
junction per quad), then the elementwise tails overlap the next quad's
matmuls. PSUM: two pools of 4 banks each (S: r/z, M: gi/gh).
"""

from contextlib import ExitStack

import ml_dtypes
import numpy as np

import concourse.bass as bass
import concourse.tile as tile
from concourse import bacc, mybir
from concourse.bass_utils import run_bass_kernel_spmd

H = 2048
B = 4096
N_CORES = 8
BL = B // N_CORES  # 512 batch rows per core
P = 128
NKB = H // P  # 16 contraction chunks of 128
ND = NKB // 2  # 8 DoubleRow chunks of 256
NNB = H // P  # 16 hidden (output) blocks
QUAD = 4
F32 = mybir.dt.float32
BF16 = mybir.dt.bfloat16
F8 = mybir.dt.float8e4

SX = 32.0  # activation quant scale
SW = 8192.0  # weight quant scale
SINV = 1.0 / (SX * SW)
F8MAX = 240.0  # TRN FP8_EXP4 max normal

# fp8 weight matrix order: 0: W_ih[0] (r)   1: W_hh[0] (r)
#                          2: W_ih[1] (z)   3: W_hh[1] (z)
#                          4: W_hh[2] (n, hh half)
# bf16 weights: W_ih[2] (n, ih half)


def _build_program() -> bacc.Bacc:
    nc = bacc.Bacc(
        "TRN2", target_bir_lowering=False, debug=False, num_devices=N_CORES
    )

    xq8 = nc.dram_tensor("xq8", [P, NKB, BL], F8, kind="ExternalInput").ap()
    hq8 = nc.dram_tensor("hq8", [P, NKB, BL], F8, kind="ExternalInput").ap()
    xb = nc.dram_tensor("xb", [P, NKB, BL], BF16, kind="ExternalInput").ap()
    hb = nc.dram_tensor("hb", [P, NKB, BL], BF16, kind="ExternalInput").ap()
    w8 = nc.dram_tensor("w8", [5, NNB, P, NKB, P], F8, kind="ExternalInput").ap()
    wb = nc.dram_tensor("wb", [NNB, P, NKB, P], BF16, kind="ExternalInput").ap()
    b = nc.dram_tensor("b", [P, 4 * NNB], F32, kind="ExternalInput").ap()
    out = nc.dram_tensor("out", [H, BL], BF16, kind="ExternalOutput").ap()

    with tile.TileContext(nc) as tc, ExitStack() as ctx:
        const = ctx.enter_context(tc.tile_pool(name="const", bufs=1))
        acts = ctx.enter_context(tc.tile_pool(name="acts", bufs=1))
        w8pool = ctx.enter_context(tc.tile_pool(name="w8pool", bufs=10))
        wbpool = ctx.enter_context(tc.tile_pool(name="wbpool", bufs=6))
        rzsave = ctx.enter_context(tc.tile_pool(name="rzsave", bufs=NNB))
        gates = ctx.enter_context(tc.tile_pool(name="gates", bufs=2))
        opool = ctx.enter_context(tc.tile_pool(name="opool", bufs=3))
        # Two PSUM pools of 4 banks: ps_a holds r (phase S) / gi (phase M),
        # ps_b holds z / gh.
        ps_a = ctx.enter_context(tc.tile_pool(name="ps_a", bufs=4, space="PSUM"))
        ps_b = ctx.enter_context(tc.tile_pool(name="ps_b", bufs=4, space="PSUM"))

        btile = const.tile([P, 4 * NNB], F32)
        nc.scalar.dma_start(btile[:], b[:])

        xq8_sb = acts.tile([P, NKB, BL], F8)
        hq8_sb = acts.tile([P, NKB, BL], F8)
        xb_sb = acts.tile([P, NKB, BL], BF16)
        hb_sb = acts.tile([P, NKB, BL], BF16)

        # Phase-S weight slabs, need-ordered on the sync ring. nb0's r/z-ih
        # slabs chase xq8 so DoubleRow matmuls start after ~1.5 MiB.
        s8 = {}
        # xq8 on the sync ring, hq8 on the scalar ring in parallel: both
        # 1 MiB streams land ~5us sooner than serialized on one ring.
        nc.sync.dma_start(xq8_sb[:], xq8[:])
        nc.scalar.dma_start(hq8_sb[:], hq8[:])
        for m in (0, 2):
            s = w8pool.tile([P, NKB, P], F8, tag="w8", name=f"w8_{m}_0")
            nc.sync.dma_start(s[:], w8[m, 0])
            s8[(m, 0)] = s
        for m in (1, 3):
            s = w8pool.tile([P, NKB, P], F8, tag="w8", name=f"w8_{m}_0")
            nc.scalar.dma_start(s[:], w8[m, 0])
            s8[(m, 0)] = s
        # bf16 acts for phase M are interleaved into the S-phase sync
        # stream (nb 1..4) so they don't contend with the startup fp8 bytes.

        # PE warm-up while the first DMAs land (HAM clock-gate release).
        warm = const.tile([P, BL], BF16)
        nc.gpsimd.memset(warm[:], 0.0)
        p_warm = ps_b.tile([P, BL], F32, tag="p_b", name="p_warm")
        for _ in range(12):
            nc.tensor.matmul(
                p_warm[:], lhsT=warm[:, :P], rhs=warm[:], start=True, stop=True
            )

        def mm_dr(psum, slab, act_sb, start, stop):
            """fp8 DoubleRow K-sweep half: 8 matmuls, K=256 each."""
            for k2 in range(ND):
                nc.tensor.matmul(
                    psum[:],
                    lhsT=slab[:, 2 * k2 : 2 * k2 + 2, :],
                    rhs=act_sb[:, 2 * k2 : 2 * k2 + 2, :],
                    start=(start and k2 == 0),
                    stop=(stop and k2 == ND - 1),
                    perf_mode=mybir.MatmulPerfMode.DoubleRow,
                )

        def mm_bf(psum, slab, act_sb, start, stop):
            """bf16 K-sweep half: 16 matmuls, K=128 each."""
            for k in range(NKB):
                nc.tensor.matmul(
                    psum[:],
                    lhsT=slab[:, k : k + 1, :],
                    rhs=act_sb[:, k : k + 1, :],
                    start=(start and k == 0),
                    stop=(stop and k == NKB - 1),
                )

        def bias_ap(g, nb):
            return btile[:, g * NNB + nb : g * NNB + nb + 1]

        # ---- Phase S: all r/z DoubleRow matmuls, sigmoids saved as bf16.
        rs = [None] * NNB
        zs = [None] * NNB
        mact = [xb_sb, xb_sb, hb_sb, hb_sb]
        for nb in range(NNB):
            if nb > 0:
                for m in range(4):
                    s = w8pool.tile([P, NKB, P], F8, tag="w8",
                                    name=f"w8_{m}_{nb}")
                    nc.sync.dma_start(s[:], w8[m, nb])
                    s8[(m, nb)] = s
            if 1 <= nb <= 4:
                t = mact[nb - 1]
                src = xb if nb <= 2 else hb
                c = (nb - 1) % 2
                nc.sync.dma_start(
                    t[:, c * 8 : (c + 1) * 8, :], src[:, c * 8 : (c + 1) * 8, :]
                )
            p_r = ps_a.tile([P, BL], F32, tag="p_a", name=f"p_r{nb}")
            p_z = ps_b.tile([P, BL], F32, tag="p_b", name=f"p_z{nb}")
            mm_dr(p_r, s8[(0, nb)], xq8_sb, True, False)
            mm_dr(p_z, s8[(2, nb)], xq8_sb, True, False)
            mm_dr(p_r, s8[(1, nb)], hq8_sb, False, True)
            mm_dr(p_z, s8[(3, nb)], hq8_sb, False, True)
            s8.pop((0, nb)); s8.pop((1, nb)); s8.pop((2, nb)); s8.pop((3, nb))
            rs[nb] = rzsave.tile([P, BL], BF16, tag="rs", name=f"rs{nb}")
            zs[nb] = rzsave.tile([P, BL], BF16, tag="zs", name=f"zs{nb}")
            nc.scalar.activation(
                rs[nb][:], p_r[:], mybir.ActivationFunctionType.Sigmoid,
                bias=bias_ap(0, nb), scale=SINV,
            )
            nc.scalar.activation(
                zs[nb][:], p_z[:], mybir.ActivationFunctionType.Sigmoid,
                bias=bias_ap(1, nb), scale=SINV,
            )

        # ---- Phase M: quads of (4x gi bf16, 4x gh2 DoubleRow, 4x tail).
        for q0 in range(0, NNB, QUAD):
            quad = range(q0, q0 + QUAD)
            slb = {}
            s8h = {}
            for nb in quad:
                s = wbpool.tile([P, NKB, P], BF16, tag="wb", name=f"wb_{nb}")
                nc.sync.dma_start(s[:], wb[nb])
                slb[nb] = s
            for nb in quad:
                s = w8pool.tile([P, NKB, P], F8, tag="w8", name=f"w8_4_{nb}")
                nc.sync.dma_start(s[:], w8[4, nb])
                s8h[nb] = s
            p_gi = {}
            for nb in quad:
                p_gi[nb] = ps_a.tile([P, BL], F32, tag="p_a", name=f"p_gi{nb}")
                mm_bf(p_gi[nb], slb[nb], xb_sb, True, True)
            # gh2 DoubleRow + tail interleaved per block: tail(nb) overlaps
            # gh(nb+1)'s matmuls, so only the final block's tail is exposed.
            for nb in quad:
                p_gh = ps_b.tile([P, BL], F32, tag="p_b", name=f"p_gh{nb}")
                mm_dr(p_gh, s8h[nb], hq8_sb, True, True)
                last = nb == NNB - 1
                halves = 4 if last else (2 if nb >= NNB - QUAD else 1)
                CW = BL // halves
                # Tail in bf16 (2x DVE rate): u = gh2 + b_hh2 (descaled),
                # t = u*r, x = t + gi2 (f32: psum operand), n = tanh(x+b),
                # out = n + z*(hx - n).
                u_sb = gates.tile([P, BL], BF16, tag="u")
                t_sb = gates.tile([P, BL], BF16, tag="t")
                x_sb = gates.tile([P, BL], F32, tag="x")
                n_sb = gates.tile([P, BL], BF16, tag="n")
                d_sb = gates.tile([P, BL], BF16, tag="d")
                e_sb = gates.tile([P, BL], BF16, tag="e")
                o_sb = opool.tile([P, BL], BF16, tag="o")
                for h in range(halves):
                    hs = slice(h * CW, (h + 1) * CW)
                    nc.scalar.activation(
                        u_sb[:, hs], p_gh[:, hs],
                        mybir.ActivationFunctionType.Identity,
                        bias=bias_ap(3, nb), scale=SINV,
                    )
                    nc.vector.tensor_mul(t_sb[:, hs], u_sb[:, hs], rs[nb][:, hs])
                    nc.vector.tensor_add(x_sb[:, hs], t_sb[:, hs], p_gi[nb][:, hs])
                    nc.scalar.activation(
                        n_sb[:, hs], x_sb[:, hs],
                        mybir.ActivationFunctionType.Tanh,
                        bias=bias_ap(2, nb),
                    )
                    nc.vector.tensor_sub(
                        d_sb[:, hs], hb_sb[:, nb, hs], n_sb[:, hs]
                    )
                    nc.vector.tensor_mul(e_sb[:, hs], zs[nb][:, hs], d_sb[:, hs])
                    nc.vector.tensor_add(o_sb[:, hs], n_sb[:, hs], e_sb[:, hs])
                    if last:
                        nc.sync.dma_start(
                            out[nb * P : (nb + 1) * P, hs], o_sb[:, hs]
                        )
                if not last:
                    nc.gpsimd.dma_start(out[nb * P : (nb + 1) * P, :], o_sb[:])

    nc.compile()
    return nc


def _pack_inputs(input, hx, weight_ih, weight_hh, bias_ih, bias_hh):
    """Host-side shard + quantize + layout packing. Per-core input maps."""
    input = np.ascontiguousarray(np.asarray(input, dtype=np.float32))
    hx = np.ascontiguousarray(np.asarray(hx, dtype=np.float32))
    weight_ih = np.asarray(weight_ih, dtype=np.float32)
    weight_hh = np.asarray(weight_hh, dtype=np.float32)
    bias_ih = np.asarray(bias_ih, dtype=np.float32)
    bias_hh = np.asarray(bias_hh, dtype=np.float32)

    E4 = ml_dtypes.float8_e4m3

    def wpack(wm):
        # [kp, k, n] per nb: [nb, kp, k, n] = W[k*128+kp, nb*128+n]
        return wm.reshape(NKB, P, NNB, P).transpose(2, 1, 0, 3)

    ws8 = [weight_ih[0], weight_hh[0], weight_ih[1], weight_hh[1], weight_hh[2]]
    w8pack = np.ascontiguousarray(
        np.stack([wpack(np.clip(wm * SW, -F8MAX, F8MAX)) for wm in ws8])
        .astype(E4)
    )
    wbpack = np.ascontiguousarray(wpack(weight_ih[2]).astype(ml_dtypes.bfloat16))

    # bpack[p, g*16+nb] = bias_g[nb*128+p]; g: r_sum, z_sum, ih2, hh2
    bias_all = np.stack(
        [bias_ih[0] + bias_hh[0], bias_ih[1] + bias_hh[1], bias_ih[2], bias_hh[2]]
    )  # [4, H]
    bpack = np.ascontiguousarray(
        bias_all.reshape(4, NNB, P).transpose(2, 0, 1).reshape(P, 4 * NNB)
    )

    def t_pack(a, dt, scale=None):
        # [BL, H] -> [P, NKB, BL] with [p, k, m] = a[m, k*128+p]
        t = a.T.reshape(NKB, P, BL).transpose(1, 0, 2)
        if scale is not None:
            t = np.clip(t * scale, -F8MAX, F8MAX)
        return np.ascontiguousarray(t.astype(dt))

    in_maps = []
    for c in range(N_CORES):
        sl = slice(c * BL, (c + 1) * BL)
        in_maps.append(
            {
                "xq8": t_pack(input[sl], E4, SX),
                "hq8": t_pack(hx[sl], E4, SX),
                "xb": t_pack(input[sl], ml_dtypes.bfloat16),
                "hb": t_pack(hx[sl], ml_dtypes.bfloat16),
                "w8": w8pack,
                "wb": wbpack,
                "b": bpack,
            }
        )
    return in_maps


_PROGRAM_CACHE = []


def kernel(input, hx, weight_ih, weight_hh, bias_ih, bias_hh, _trace=False):
    if not _PROGRAM_CACHE:
        _PROGRAM_CACHE.append(_build_program())
    nc = _PROGRAM_CACHE[0]
    in_maps = _pack_inputs(input, hx, weight_ih, weight_hh, bias_ih, bias_hh)
    res = run_bass_kernel_spmd(nc, in_maps, list(range(N_CORES)), trace=_trace)
    out = np.empty((B, H), dtype=np.float32)
    for c in range(N_CORES):
        out[c * BL : (c + 1) * BL] = res.results[c]["out"].T.astype(np.float32)
    if _trace:
        kernel.last_exec_time_ns = res.exec_time_ns
    return out
